# revision 7
# baseline (speedup 1.0000x reference)
"""Trainium2 Bass kernel for the DGCNN-style message-passing block.

Math (per batch b, data-parallel over 8 cores):
    proj = x @ Wp^T
    m[i] = max_k proj[knn[i,k]]           (edge maxpool: max_k(f_j - f_i) = m_i - proj_i)
    x1 = BN_l((m - proj) @ Wl^T);  x2 = BN_g(x @ Wg^T)
    h  = BN_1((x1+x2) @ W1^T + b1); a = sigmoid(BN_2(h @ W2^T + b2))
    out = BN_f(a*x1 + (1-a)*x2)

All BNs are inference-affine and fold into the weights host-side, and proj
composes into the local branch (x1 = m@Wl'^T - x@(Wl'Wp)^T + t_l), giving:
    f  = m@Wmf^T + x@Wxf^T + tf          (= x1+x2, feeds h)
    d' = m@Wmd^T + x@Wxd^T + td          (= s_f*(x1-x2))
    v  = x@Wxv^T + tv                    (= BN_f(x2))
    h  = f@W1'^T + t1;  a = sigmoid(h@W2'^T + t2)
    out = v + a*d'

Everything runs in bf16 (PSUM accumulation stays f32).  proj never leaves
SBUF: the KNN gather uses SBUF-source transposed dma_gather (node n lives at
partition n%128, rank n//128 of proj_sb — the natural layout the proj matmul
produces), which deposits gathered neighbor rows ALREADY feature-major
([128ch, CK, n_idx]), so the old DRAM round-trip, the PE transposes and their
PSUM evacuations all disappear.  Transpose-mode gathers carry at most 896
indices (128 in-flight descriptor cap, 2 descs per 16 idx + 2), so nodes are
processed in four 896-node groups (one gather per k) plus a 512-node tail
whose 16x512 edge stream is packed k-major into 10 ops of 896/128 idx; the
tail's per-k max pieces are accumulated in-place with column slices as each
op lands.  Desc-gen is 994ns fixed + 0.34ns/idx per op on Pool: 74 ops total
= ~96us of Pool, just under the ~99us of serialized DMA (gather 93us + out
writes), so the stream stays DMA-paced.  Max chains run on DVE in
feature-major layout; stage_b (f/h/d/a matmul passes + out) is pipelined one
group behind the gather stream, as before.
"""

import numpy as np
import ml_dtypes

import concourse.bass as bass
import concourse.mybir as mybir
import concourse.tile as tile
from concourse import bacc
from concourse.bass_utils import run_bass_kernel_spmd

F32 = mybir.dt.float32
BF16 = mybir.dt.bfloat16
I16 = mybir.dt.int16

B, N, K, C = 8, 4096, 16, 256
P = 128
NT = N // P          # 32 node stripes
CK = C // P          # 2 channel chunks
EPS = 1e-5

GW = 896             # main node-group width (transpose-gather idx cap)
TAILW = N - 4 * GW   # 512-node tail group
GROUPS = [(0, GW), (GW, GW), (2 * GW, GW), (3 * GW, GW), (4 * GW, TAILW)]
NGRP = len(GROUPS)
# tail: 16*TAILW edges packed k-major, split into ops of <=896 idx
TAIL_EDGES = K * TAILW
TAIL_OPS = []
_o = 0
while _o < TAIL_EDGES:
    TAIL_OPS.append((_o, min(896, TAIL_EDGES - _o)))
    _o += 896
KCOLS = K * N // 16  # total idx columns (16-partition wrap)

AF = mybir.ActivationFunctionType

_phase_cb = None  # analysis hook: called with a label at phase boundaries


def _mark(label):
    if _phase_cb is not None:
        _phase_cb(label)


def build_bass(n_cores: int = 8, reps: int = 1):
    nc = bacc.Bacc(
        "TRN2",
        target_bir_lowering=False,
        debug=False,
        enable_asserts=False,
        num_devices=n_cores,
        num_swdge_queues=4,
    )

    xT = nc.dram_tensor("xT", [C, N], BF16, kind="ExternalInput").ap()
    knn_i = nc.dram_tensor("knn_i", [P, KCOLS], I16, kind="ExternalInput").ap()
    # packed weights: [128, (w, kc, 256)] bf16; order: wpT,wxf,wxd,wxv,wmf,wmd,w1,w2
    wb = nc.dram_tensor("wb", [P, 8 * CK * C], BF16, kind="ExternalInput").ap()
    bias = nc.dram_tensor("bias", [P, 10], F32, kind="ExternalInput").ap()
    outT = nc.dram_tensor("outT", [C, N], BF16, kind="ExternalOutput").ap()

    with tile.TileContext(nc) as tc:
        for _ in range(reps):
            kernel_body(tc, xT, knn_i, wb, bias, outT)
    nc.compile()
    return nc


def kernel_body(tc, xT, knn_i, wb, bias, outT):
    nc = tc.nc

    with (
        tc.tile_pool(name="const", bufs=1) as cpool,
        tc.tile_pool(name="projp", bufs=1) as projp,
        tc.tile_pool(name="gat", bufs=1) as gat,
        tc.tile_pool(name="mt", bufs=3) as mtp,
        tc.tile_pool(name="units", bufs=2) as up,
        tc.tile_pool(name="outp", bufs=3) as outp,
        tc.tile_pool(name="psA", bufs=5, space="PSUM") as psA,
        tc.tile_pool(name="psNP", bufs=3, space="PSUM") as psNP,
    ):
        # ---- load order: wpT (proj weights), group-0 indices, xT chunks;
        # the rest of the weights / bias / later indices are issued after the
        # proj head so they don't delay the first gathers.
        wb_sb = cpool.tile([P, 8 * CK * C], BF16)
        nc.sync.dma_start(wb_sb[:, :CK * C], wb[:, :CK * C])          # wpT first

        kidx_all = cpool.tile([P, KCOLS], I16)
        # group g's idx block starts at col goff[g]; per-k sub-blocks of
        # GW/16 cols for the main groups; tail block is one k-major stream.
        goff = [g * K * GW // 16 for g in range(4)] + [4 * K * GW // 16]

        nc.sync.dma_start(kidx_all[:, :goff[1]], knn_i[:, :goff[1]])

        # x^T as two per-kc tiles spanning all nodes (chunked loads)
        xt = []
        for kc in range(CK):
            t = cpool.tile([P, N], BF16, name=f"xt{kc}")
            xt.append(t)
        for cc in range(4):
            for kc in range(CK):
                nc.sync.dma_start(
                    xt[kc][:, cc * 1024:(cc + 1) * 1024],
                    xT[kc * P:(kc + 1) * P, cc * 1024:(cc + 1) * 1024])

        def wslice(w_i):
            return [wb_sb[:, (w_i * CK + kc) * C:(w_i * CK + kc + 1) * C] for kc in range(CK)]

        wpT_sb = wslice(0)
        wxf_sb = wslice(1)
        wxd_sb = wslice(2)
        wxv_sb = wslice(3)
        wmf_sb = wslice(4)
        wmd_sb = wslice(5)
        w1_sb = wslice(6)
        w2_sb = wslice(7)

        bias_sb = cpool.tile([P, 10], F32)

        # ---- phase 1: proj -> SBUF only (node n -> partition n%128, rank
        # n//128), pipelined 2 node-stripes per PSUM buffer.
        proj_sb = projp.tile([P, NT, C], BF16)
        _mark("head")
        for tp in range(NT // 2):
            ps = psNP.tile([P, 2, C], F32, name="ps_np", tag="ps_np")
            for j in range(2):
                t = 2 * tp + j
                nc.tensor.matmul(ps[:, j, :], lhsT=xt[0][:, t * P:(t + 1) * P],
                                 rhs=wpT_sb[0], start=True, stop=False,
                                 skip_group_check=True)
                nc.tensor.matmul(ps[:, j, :], lhsT=xt[1][:, t * P:(t + 1) * P],
                                 rhs=wpT_sb[1], start=False, stop=True,
                                 skip_group_check=True)
            if tp % 2 == 0:
                nc.scalar.activation(proj_sb[:, 2 * tp:2 * tp + 2, :], ps[:], AF.Copy)
            else:
                nc.vector.tensor_copy(proj_sb[:, 2 * tp:2 * tp + 2, :], ps[:])

        # deferred loads (slot into the gather stream)
        nc.sync.dma_start(wb_sb[:, CK * C:], wb[:, CK * C:])
        nc.sync.dma_start(bias_sb[:], bias[:])
        nc.sync.dma_start(kidx_all[:, goff[1]:], knn_i[:, goff[1]:])

        proj_src = proj_sb[:]  # [128, NT*C] SBUF source for gathers

        def gather(out_ap, idx_ap, nidx, q):
            nc.gpsimd.dma_gather(
                out_ap=out_ap, in_ap=proj_src, idxs_ap=idx_ap,
                num_idxs=nidx, num_idxs_reg=nidx, elem_size=C,
                transpose=True, queue_num=q,
                sbuf_tokens_per_rank=P,
                sbuf_free_dim_per_rank=C * 2,
            )

        # ---- phases 2-4: software-pipelined per node-group ----
        # stage A(g): gathers (feature-major out) + DVE max chains
        # stage B(g): f/d/h/a matmul passes + out, in node slices

        def evac(engine, out_ap, ps, bcol_mc, func=AF.Identity):
            """PSUM -> SBUF with per-channel bias; Act or DVE."""
            if engine == "act" or func != AF.Identity:
                nc.scalar.activation(out_ap, ps[:], func,
                                     bias=bias_sb[:, bcol_mc:bcol_mc + 1],
                                     scale=1.0)
            else:
                nc.vector.tensor_scalar(
                    out=out_ap, in0=ps[:],
                    scalar1=bias_sb[:, bcol_mc:bcol_mc + 1], scalar2=None,
                    op0=mybir.AluOpType.add)

        def mx_pass(n0, mt, sl, out_sb, wm_sb, wx_sb, bcol, mc, eng="act",
                    func=AF.Identity):
            """psum = [m-part] + [x-part] over a node slice; evac w/ bias."""
            ps = psA.tile([P, sl.stop - sl.start], F32, name="ps_fp", tag="ps_fp")
            first = True
            if wm_sb is not None:
                for kc in range(CK):
                    nc.tensor.matmul(
                        ps[:], lhsT=wm_sb[kc][:, mc * P:(mc + 1) * P],
                        rhs=mt[:, kc, sl], start=first and kc == 0,
                        stop=False, skip_group_check=True)
                first = False
            for kc in range(CK):
                last = kc == CK - 1
                nc.tensor.matmul(
                    ps[:], lhsT=wx_sb[kc][:, mc * P:(mc + 1) * P],
                    rhs=xt[kc][:, n0 + sl.start:n0 + sl.stop],
                    start=first and kc == 0, stop=last,
                    skip_group_check=True)
            evac(eng, out_sb[:, mc, sl], ps, bcol + mc, func)

        def hx_pass(w_sb, in_sb, sl, out_sb, bcol, mc, eng="act",
                    func=AF.Identity):
            ps = psA.tile([P, sl.stop - sl.start], F32, name="ps_fp", tag="ps_fp")
            for kc in range(CK):
                nc.tensor.matmul(
                    ps[:], lhsT=w_sb[kc][:, mc * P:(mc + 1) * P],
                    rhs=in_sb[:, kc, sl],
                    start=kc == 0, stop=kc == CK - 1,
                    skip_group_check=True)
            evac(eng, out_sb[:, mc, sl], ps, bcol + mc, func)

        vg = [None] * NGRP

        def stage_a_main(g):
            """896-node group: one gather per k + two DVE max chains."""
            _mark(f"a{g}")
            n0, sz = GROUPS[g]
            cols = GW // 16
            # v = x@wxv (x-only; hoisted so stage_b's tail is shorter)
            v_sb = up.tile([P, CK, GW], BF16, name="v_sb", tag="v")
            vg[g] = v_sb
            for sl in (slice(0, 448), slice(448, 896)):
                for mc in range(CK):
                    mx_pass(n0, None, sl, v_sb, None, wxv_sb, 4, mc, eng="act")

            gk = []
            for k in range(K):
                gtl = gat.tile([P, CK, GW], BF16, name=f"g_{g}_{k}",
                               tag="gk", bufs=20)
                lo = goff[g] + k * cols
                gather(gtl[:], kidx_all[:, lo:lo + cols], GW, k % 4)
                gk.append(gtl[:])
            accA = gat.tile([P, CK, GW], BF16, name=f"accA{g}", tag="accA", bufs=2)
            accB = gat.tile([P, CK, GW], BF16, name=f"accB{g}", tag="accB", bufs=2)
            nc.vector.tensor_tensor(out=accA[:], in0=gk[0], in1=gk[1],
                                    op=mybir.AluOpType.max)
            for k in range(2, K // 2):
                nc.vector.tensor_tensor(out=accA[:], in0=accA[:], in1=gk[k],
                                        op=mybir.AluOpType.max)
            nc.vector.tensor_tensor(out=accB[:], in0=gk[K // 2],
                                    in1=gk[K // 2 + 1],
                                    op=mybir.AluOpType.max)
            for k in range(K // 2 + 2, K):
                nc.vector.tensor_tensor(out=accB[:], in0=accB[:], in1=gk[k],
                                        op=mybir.AluOpType.max)
            mt = mtp.tile([P, CK, GW], BF16, name="mt", tag="mt")
            nc.vector.tensor_tensor(out=mt[:], in0=accA[:], in1=accB[:],
                                    op=mybir.AluOpType.max)
            return mt

        def stage_a_tail(g):
            """512-node tail: 16x512 k-major edge stream in 10 ops; per-k
            column pieces max-accumulated in place as each op lands."""
            _mark(f"at{g}")
            n0, sz = GROUPS[g]
            base = goff[4]
            tiles = []
            for i, (off, nidx) in enumerate(TAIL_OPS):
                if nidx == GW:
                    gtl = gat.tile([P, CK, GW], BF16, name=f"t_{i}", tag="gk",
                                   bufs=20)
                else:
                    gtl = gat.tile([P, CK, nidx], BF16, name=f"t_{i}",
                                   tag="gks", bufs=1)
                gather(gtl[:], kidx_all[:, base + off // 16:
                                        base + (off + nidx) // 16],
                       nidx, i % 4)
                tiles.append((off, nidx, gtl))
            # mt accumulates per-k pieces: edge e = k*sz + j -> node j
            mt = mtp.tile([P, CK, GW], BF16, name="mt", tag="mt")
            mtv = mt[:, :, :sz]
            # pieces[k] = list of (j0, len, tile, tile_off)
            pieces = [[] for _ in range(K)]
            for off, nidx, gtl in tiles:
                e = off
                while e < off + nidx:
                    k = e // sz
                    j0 = e % sz
                    ln = min((k + 1) * sz, off + nidx) - e
                    pieces[k].append((j0, ln, gtl, e - off))
                    e += ln
            # initialize mt from k=0's piece(s), then max-accumulate the rest
            for j0, ln, gtl, to in pieces[0]:
                nc.vector.tensor_copy(mtv[:, :, j0:j0 + ln],
                                      gtl[:, :, to:to + ln])
            for k in range(1, K):
                for j0, ln, gtl, to in pieces[k]:
                    nc.vector.tensor_tensor(
                        out=mtv[:, :, j0:j0 + ln], in0=mtv[:, :, j0:j0 + ln],
                        in1=gtl[:, :, to:to + ln], op=mybir.AluOpType.max)
            # v pass for the tail
            v_sb = up.tile([P, CK, GW], BF16, name="v_sb", tag="v")
            vg[g] = v_sb
            for mc in range(CK):
                mx_pass(n0, None, slice(0, sz), v_sb, None, wxv_sb, 4, mc,
                        eng="act")
            return mtv

        def stage_b(g, mt, split_evac=False, ws=448):
            _mark(f"b{g}")
            n0, sz = GROUPS[g]
            eng2 = "dve" if split_evac else "act"
            sls = [slice(i * ws, min((i + 1) * ws, sz))
                   for i in range((sz + ws - 1) // ws)]
            f_sb = up.tile([P, CK, GW], BF16, name="f_sb", tag="f")
            d_sb = up.tile([P, CK, GW], BF16, name="d_sb", tag="d")
            h_sb = up.tile([P, CK, GW], BF16, name="h_sb", tag="h")
            a_sb = up.tile([P, CK, GW], BF16, name="a_sb", tag="a")
            v_sb = vg[g]
            # PE order f,f,d,d,h,h,a,a: f and d depend only on mt/xt, so PE
            # gets a long uninterrupted run (p-state ramp) while f's evacs
            # complete for h.
            for sl in sls:
                for mc in range(CK):
                    mx_pass(n0, mt, sl, f_sb, wmf_sb, wxf_sb, 0, mc,
                            eng="act" if mc == 0 else eng2)
            for sl in sls:
                for mc in range(CK):
                    mx_pass(n0, mt, sl, d_sb, wmd_sb, wxd_sb, 2, mc,
                            eng="act" if mc == 0 else eng2)
            for sl in sls:
                for mc in range(CK):
                    hx_pass(w1_sb, f_sb, sl, h_sb, 6, mc,
                            eng="act" if mc == 0 else eng2)
            for sl in sls:
                for mc in range(CK):
                    hx_pass(w2_sb, h_sb, sl, a_sb, 8, mc, eng="act",
                            func=AF.Sigmoid)
            for sl in sls:
                # out = v + a*d'  (bf16, op pair + one DMA)
                ot = outp.tile([P, CK, sl.stop - sl.start], BF16,
                               name="ot", tag="ot")
                nc.vector.tensor_tensor(out=ot[:], in0=a_sb[:, :, sl],
                                        in1=d_sb[:, :, sl],
                                        op=mybir.AluOpType.mult)
                nc.vector.tensor_tensor(out=ot[:], in0=ot[:], in1=v_sb[:, :, sl],
                                        op=mybir.AluOpType.add)
                nn = n0 + sl.start
                nc.sync.dma_start(
                    outT[:, nn:nn + (sl.stop - sl.start)].rearrange(
                        "(k p) n -> p k n", p=P),
                    ot[:])

        mt_prev = stage_a_main(0)
        for g in range(1, NGRP):
            mt_g = stage_a_tail(g) if g == NGRP - 1 else stage_a_main(g)
            stage_b(g - 1, mt_prev)
            mt_prev = mt_g
        stage_b(NGRP - 1, mt_prev, split_evac=True, ws=256)


# ---------------- host side ----------------

def _fold(proj_W, local_W, glob_W, aff_W1, aff_b1, aff_W2, aff_b2,
          bn_local, bn_glob, bn_aff1, bn_aff2, bn_final):
    f32 = np.float32

    def bn_st(p):
        p = np.asarray(p, f32)
        g, b, m, v = p
        s = g / np.sqrt(v + EPS)
        return s.astype(f32), (b - m * s).astype(f32)

    Wp = np.asarray(proj_W, f32)
    s_l, t_l = bn_st(bn_local)
    s_g, t_g = bn_st(bn_glob)
    s_1, t_1 = bn_st(bn_aff1)
    s_2, t_2 = bn_st(bn_aff2)
    s_f, t_f = bn_st(bn_final)

    Wlp = s_l[:, None] * np.asarray(local_W, f32)
    Wgp = s_g[:, None] * np.asarray(glob_W, f32)
    Wlproj = (Wlp @ Wp).astype(f32)

    def pack(ws, dt):
        # ws: list of [C, C] W^T arrays -> [128, n*CK*C]
        P_, CK_ = 128, 2
        out = np.zeros((P_, len(ws) * CK_ * 256), dt)
        for w_i, m in enumerate(ws):
            for kc in range(CK_):
                out[:, (w_i * CK_ + kc) * 256:(w_i * CK_ + kc + 1) * 256] = m[kc * P_:(kc + 1) * P_, :].astype(dt)
        return out

    w = {}
    wpT = np.ascontiguousarray(Wp.T)
    wxf = np.ascontiguousarray((Wgp - Wlproj).T)
    wxd = np.ascontiguousarray((-s_f[:, None] * (Wlproj + Wgp)).T)
    wxv = np.ascontiguousarray((s_f[:, None] * Wgp).T)
    wmf = np.ascontiguousarray(Wlp.T)
    wmd = np.ascontiguousarray((s_f[:, None] * Wlp).T)
    w1 = np.ascontiguousarray((s_1[:, None] * np.asarray(aff_W1, f32)).T)
    w2 = np.ascontiguousarray((s_2[:, None] * np.asarray(aff_W2, f32)).T)
    w["wb"] = pack([wpT, wxf, wxd, wxv, wmf, wmd, w1, w2], ml_dtypes.bfloat16)

    tf = t_l + t_g
    td = s_f * (t_l - t_g)
    tv = s_f * t_g + t_f
    t1 = s_1 * np.asarray(aff_b1, f32) + t_1
    t2 = s_2 * np.asarray(aff_b2, f32) + t_2
    # bias[p, 2*j + mc] = coeff_j[mc*128 + p]
    bias = np.zeros((P, 10), f32)
    for j, tt in enumerate((tf, td, tv, t1, t2)):
        for mc in range(CK):
            bias[:, 2 * j + mc] = tt[mc * P:(mc + 1) * P]
    w["bias"] = bias
    return w


def _wrap16(flat):
    """[M] int16 edge stream -> [128, M/16] wrapped (i -> [i%16, i//16]),
    replicated 8x across the 16-partition groups."""
    M = flat.shape[-1]
    blk = flat.reshape(M // 16, 16).T          # [16, M/16]
    return np.tile(blk, (8, 1)).astype(np.int16)


_NC_CACHE = {}


def _get_nc():
    if "nc" not in _NC_CACHE:
        _NC_CACHE["nc"] = build_bass(B)
    return _NC_CACHE["nc"]


def kernel(**inputs) -> np.ndarray:
    x = np.ascontiguousarray(np.asarray(inputs["x"], np.float32))      # [B,N,C]
    knn = np.asarray(inputs["knn"]).astype(np.int64)                   # [B,N,K]
    w = _fold(
        inputs["proj_W"], inputs["local_W"], inputs["glob_W"],
        inputs["aff_W1"], inputs["aff_b1"], inputs["aff_W2"], inputs["aff_b2"],
        inputs["bn_local"], inputs["bn_glob"], inputs["bn_aff1"],
        inputs["bn_aff2"], inputs["bn_final"],
    )

    # SBUF-source gather: index = raw node id (partition n%128, rank n//128)
    r = knn.astype(np.int16)                                           # [B,N,K]
    blocks = []
    for n0, sz in GROUPS[:4]:
        ids = r[:, n0:n0 + sz, :]                                      # [B,sz,K]
        # per-k blocks of GW idx, k-major
        kmaj = ids.transpose(0, 2, 1).reshape(B, K * sz)               # [B,K*sz]
        blocks.append(np.stack([_wrap16(kmaj[b]) for b in range(B)]))
    n0, sz = GROUPS[4]
    ids = r[:, n0:n0 + sz, :].transpose(0, 2, 1).reshape(B, K * sz)    # k-major
    blocks.append(np.stack([_wrap16(ids[b]) for b in range(B)]))
    ridx = np.concatenate(blocks, axis=2).astype(np.int16)             # [B,128,KCOLS]

    nc = _get_nc()
    in_maps = []
    for b in range(B):
        m = {"xT": np.ascontiguousarray(x[b].T).astype(ml_dtypes.bfloat16),
             "knn_i": np.ascontiguousarray(ridx[b])}
        for k2, v in w.items():
            m[k2] = v
        in_maps.append(m)

    res = run_bass_kernel_spmd(nc, in_maps, core_ids=list(range(B)))
    out = np.stack([res.results[b]["outT"].astype(np.float32).T for b in range(B)])
    return out.astype(np.float32)


if __name__ == "__main__":
    nc = build_bass(1)
    print("built OK")
